# revision 34
# baseline (speedup 1.0000x reference)
"""GCN layer (gather + segment_sum + linear + relu) on 8 TRN2 NeuronCores.

v2 strategy (dst-partitioned, DMA-gather + narrow-window TensorE aggregation):
  - Nodes split into 8 ranges of 6250; core i owns edges whose dst is in its
    range and produces those output rows.
  - Per core, dsts are assigned by a load balancer to cells of <= 63 dsts
    (8 cells = one 512-wide PSUM chunk holding h^T [128 feat x 512 slot]
    f32). Balancing makes the per-(cell, src-half) edge counts nearly equal
    so the baked SPMD row quota (max over cores, rounded to 128 - PE matmul
    tiles must start at base partition 0, non-zero bases fault) wastes ~6%.
  - Edges sorted by (chunk, half, cell, slot). One dma_gather per
    (chunk, half) slab pulls the rows (fp16 table in 2 halves for int16
    indices) into msgs [128, cols, 128]; pad rows gather row 0 and carry a
    sentinel slot so their one-hot column is zero.
  - One-hots for a slab are built in one DVE tensor_tensor
    (iota[64] == slot), [128, ntiles, 64] fp16.
  - Aggregation: per 128-row tile, psum[:, cell*64 : +64] += msgs^T @ onehot
    (PSUM pre-zeroed by a [1x128]@[1x512] zero matmul; PE base partitions
    stay 0 - non-zero bases fault on HW).
  - Epilogue per chunk: Act copies h^T to SBUF fp16; per 128-slot block
    out = relu(h @ W + b) via two matmuls + Act Relu; DMA out fp16 to slot
    space [6656, 128]; the host unpermutes slots -> dsts and upcasts to f32.
The gather (~106k random 256B rows/core) is the bottleneck: dma_gather is
bound by the 4 SWDGE queues (~2ns/row measured; queue count scales it
linearly and ucode caps at 4). DVE/PE/Act work is batched into slab-sized
instructions so the gather stream never stalls on compute.
"""

import numpy as np

import concourse.mybir as mybir
import concourse.tile as tile
from concourse import bacc
from concourse.bass_utils import run_bass_kernel_spmd

P = 128
CHUNK = 512
CELLW = 64  # psum window per cell
CELLD = 63  # max dsts per cell
NCELL = CHUNK // CELLW  # cells per chunk
SENT = 600.0  # one-hot miss sentinel (exact in fp16, > CELLW)
SORT_SRC = False  # src-sorted rows measured no better than slot order


class Cfg:
    def __init__(self, n_nodes=50000, n_edges=800000, d=128, n_cores=8):
        self.n_nodes = n_nodes
        self.n_edges = n_edges
        self.d = d
        self.n_cores = n_cores
        self.npc = n_nodes // n_cores
        assert self.npc * n_cores == n_nodes
        self.ncells = -(-self.npc // CELLD)
        self.ncells = -(-self.ncells // NCELL) * NCELL  # fill whole chunks
        self.nchunks = self.ncells // NCELL
        self.nslots = self.ncells * CELLW
        self.half = (n_nodes + 1) // 2  # feature-table row split (int16 idx)
        assert self.half < 32768


CFG = Cfg()


def _balance(h0, h1, ncells, cap=508):
    """Assign dsts (with per-half edge counts h0/h1) to ncells cells of
    <= CELLD dsts, greedily minimizing the max relative per-half cell load.
    The last ncells//8 cells get a 384-row cap (shared across cores) so
    their baked quota rounds to 384 instead of 512, trimming gathered pad
    rows by ~3%."""
    npc = len(h0)
    n_light = ncells // 8
    caps = np.full(ncells, 512.0)
    if n_light:
        caps[-n_light:] = 384.0
    order = np.argsort(-(h0 + h1), kind="stable")
    s0 = np.zeros(ncells)
    s1 = np.zeros(ncells)
    cnt = np.zeros(ncells, np.int64)
    cell_of = np.zeros(npc, np.int64)
    for d in order:
        cand = np.maximum((s0 + h0[d]) / caps, (s1 + h1[d]) / caps)
        cand[cnt >= CELLD] = np.inf
        k = int(np.argmin(cand))
        cell_of[d] = k
        s0[k] += h0[d]
        s1[k] += h1[d]
        cnt[k] += 1
    # slot = cell*CELLW + rank within cell (by dst id)
    slot = np.zeros(npc, np.int64)
    for k in range(ncells):
        members = np.where(cell_of == k)[0]
        slot[members] = k * CELLW + np.arange(len(members))
    return slot


class Plan:
    """Baked SPMD structure (identical across cores). All tiles are full
    128-row, base-partition 0."""

    def __init__(self, cfg, quota_raw, slots):
        self.cfg = cfg
        self.slots = slots  # [n_cores, npc] dst -> slot
        quota = ((np.asarray(quota_raw) + P - 1) // P) * P  # [ncells, 2]
        self.quota = quota
        nch = cfg.nchunks
        self.slab_off = np.zeros((nch, 2), np.int64)
        self.slab_rows = np.zeros((nch, 2), np.int64)
        self.cell_off = np.zeros((cfg.ncells, 2), np.int64)
        self.tiles = []  # (chunk, half, col, psum_base, tidx)
        self.slab_tiles = {}
        r = 0
        for c in range(nch):
            for h in range(2):
                self.slab_off[c, h] = r
                t_lo = len(self.tiles)
                for ci in range(NCELL):
                    cell = c * NCELL + ci
                    self.cell_off[cell, h] = r
                    q = int(quota[cell, h])
                    assert q % P == 0
                    for k in range(q // P):
                        col = (r - self.slab_off[c, h]) // P
                        self.tiles.append(
                            (c, h, col, ci * CELLW, len(self.tiles))
                        )
                        r += P
                self.slab_rows[c, h] = r - self.slab_off[c, h]
                self.slab_tiles[(c, h)] = (t_lo, len(self.tiles))
        self.total_rows = r
        self.ntiles = len(self.tiles)

    def key(self):
        return (
            self.cfg.n_nodes,
            self.cfg.n_edges,
            self.total_rows,
            tuple(self.quota.reshape(-1).tolist()),
        )


def _prepare(cfg, edge_src, edge_dst):
    npc = cfg.npc
    core = edge_dst // npc
    dloc = (edge_dst - core * npc).astype(np.int64)
    half = (edge_src >= cfg.half).astype(np.int64)
    sloc = (edge_src - half * cfg.half).astype(np.int16)

    # per-(core, dst) half counts for balancing
    hc = np.zeros((2, cfg.n_nodes), np.int64)
    np.add.at(hc[0], edge_dst[half == 0], 1)
    np.add.at(hc[1], edge_dst[half == 1], 1)
    slots = np.zeros((cfg.n_cores, npc), np.int64)
    for i in range(cfg.n_cores):
        lo, hi = i * npc, (i + 1) * npc
        slots[i] = _balance(hc[0, lo:hi], hc[1, lo:hi], cfg.ncells)

    eslot = slots[core, dloc]
    ecell = eslot >> 6
    ej = (eslot & (CELLW - 1)).astype(np.float32)
    echunk = ecell // NCELL
    ecic = ecell % NCELL
    # row layout: (core, chunk, half, cell-in-chunk, src index). Sorting by
    # src within each cell makes every 16-row gather descriptor read
    # ascending, clustered table addresses (DRAM locality); the one-hot
    # carries each row's dst slot so row order within a cell is free.
    key = ((core * cfg.nchunks + echunk) * 2 + half) * NCELL + ecic
    order = np.lexsort(
        (sloc.astype(np.int32) if SORT_SRC else eslot, key)
    )
    sloc, ej, key = sloc[order], ej[order], key[order]
    nkeys = cfg.n_cores * cfg.nchunks * 2 * NCELL
    counts = np.bincount(key, minlength=nkeys).reshape(
        cfg.n_cores, cfg.nchunks, 2, NCELL
    )
    # quota indexed [ncells, 2]
    cmax = counts.max(axis=0)  # [nchunks, 2, NCELL]
    quota_raw = np.transpose(cmax, (0, 2, 1)).reshape(cfg.ncells, 2)
    plan = Plan(cfg, quota_raw, slots)

    starts = np.concatenate([[0], np.cumsum(counts.reshape(-1))])
    idx16 = np.zeros((cfg.n_cores, P, plan.total_rows // 16), np.int16)
    dstw = np.full((cfg.n_cores, P, plan.ntiles), SENT, np.float16)
    for i in range(cfg.n_cores):
        idx_rows = np.zeros(plan.total_rows, np.int16)
        dst_rows = np.full(plan.total_rows, SENT, np.float32)
        for c in range(cfg.nchunks):
            for h in range(2):
                for ci in range(NCELL):
                    g = ((i * cfg.nchunks + c) * 2 + h) * NCELL + ci
                    n = counts.reshape(-1)[g]
                    if n == 0:
                        continue
                    o = plan.cell_off[c * NCELL + ci, h]
                    s = starts[g]
                    idx_rows[o : o + n] = sloc[s : s + n]
                    dst_rows[o : o + n] = ej[s : s + n]
        idx16[i] = np.tile(idx_rows.reshape(plan.total_rows // 16, 16).T, (8, 1))
        for (c, h, col, base, t) in plan.tiles:
            g0 = plan.slab_off[c, h] + col * P
            dstw[i, :, t] = dst_rows[g0 : g0 + P].astype(np.float16)
    return plan, idx16, dstw


def _build(cfg, plan, nq=4, repeat=1, mode="full", loop=False, gsplit=2, single_packet=False):
    f16 = mybir.dt.float16
    f32 = mybir.dt.float32
    is_equal = mybir.AluOpType.is_equal
    nch = cfg.nchunks

    nc = bacc.Bacc(None, target_bir_lowering=False, num_swdge_queues=nq)
    trip = None
    if loop:
        trip = nc.declare_dram_parameter("trip", [1, 1], mybir.dt.int32, False)
    feat0 = nc.declare_dram_parameter("feat0", [cfg.half, cfg.d], f16, False)
    feat1 = nc.declare_dram_parameter(
        "feat1", [cfg.n_nodes - cfg.half, cfg.d], f16, False
    )
    idx = nc.declare_dram_parameter(
        "idx16", [P, plan.total_rows // 16], mybir.dt.int16, False
    )
    dstw = nc.declare_dram_parameter("dstw", [P, plan.ntiles], f16, False)
    iota = nc.declare_dram_parameter("iota64", [P, CELLW], f16, False)
    wmat = nc.declare_dram_parameter("wmat", [cfg.d, cfg.d], f16, False)
    bvec = nc.declare_dram_parameter("bvec", [1, cfg.d], f16, False)
    out = nc.declare_dram_parameter("out", [cfg.nslots, cfg.d], f16, True)

    with tile.TileContext(nc) as tc:
        with (
            tc.tile_pool(name="const", bufs=1) as cpool,
            tc.tile_pool(name="msgs", bufs=8) as mpool,
            tc.tile_pool(name="oh", bufs=6) as ohpool,
            tc.tile_pool(name="ep", bufs=4) as eppool,
            tc.tile_pool(name="psA", bufs=3, space="PSUM") as psa,
            tc.tile_pool(name="psB", bufs=2, space="PSUM") as psb,
        ):
            idx_sb = cpool.tile([P, plan.total_rows // 16], mybir.dt.int16)
            nc.sync.dma_start(idx_sb[:], idx[:])
            dst_sb = cpool.tile([P, plan.ntiles], f16)
            nc.sync.dma_start(dst_sb[:], dstw[:])
            iota_sb = cpool.tile([P, CELLW], f16)
            nc.sync.dma_start(iota_sb[:], iota[:])
            w_sb = cpool.tile([cfg.d, cfg.d], f16)
            nc.sync.dma_start(w_sb[:], wmat[:])
            b_sb = cpool.tile([1, cfg.d], f16)
            nc.sync.dma_start(b_sb[:], bvec[:])
            ones_sb = cpool.tile([1, cfg.d], f16)
            nc.vector.memset(ones_sb[:], 1.0)
            zrow = cpool.tile([1, cfg.d], f16)
            nc.vector.memset(zrow[:], 0.0)
            z512 = cpool.tile([1, CHUNK], f16)
            nc.vector.memset(z512[:], 0.0)

            max_cols = int(
                max((plan.slab_rows[c, h] + P - 1) // P for c in range(nch) for h in range(2))
            )
            max_nt = max(
                plan.slab_tiles[(c, h)][1] - plan.slab_tiles[(c, h)][0]
                for c in range(nch)
                for h in range(2)
            )
            msgs0 = oh0 = None
            if mode == "nogather":
                msgs0 = cpool.tile([P, max_cols, cfg.d], f16)
                nc.vector.memset(msgs0[:], 0.0)
            if mode == "constoh":
                oh0 = cpool.tile([P, max_nt, CELLW], f16)
                nc.vector.memset(oh0[:], 0.0)

            trip_val = None
            if loop:
                trip_sb = cpool.tile([1, 1], mybir.dt.int32)
                nc.sync.dma_start(trip_sb[:], trip[:])
                regs = nc.alloc_registers("trip_regs")
                for reg in regs.handles:
                    nc.engines[reg.engine].reg_load(reg, trip_sb[:1, :1])
                trip_val = nc.snap(regs, donate=True, min_val=0, max_val=1 << 20)

            gcount = 0

            def emit_rep():
                nonlocal gcount
                for c in range(nch):
                    slabs = {}
                    for h in range(2):
                        rows = int(plan.slab_rows[c, h])
                        if rows == 0:
                            continue
                        ncols = (rows + P - 1) // P
                        if mode == "nogather":
                            slabs[h] = msgs0
                            continue
                        msgs = mpool.tile([P, ncols, cfg.d], f16, tag="m")
                        s_off = int(plan.slab_off[c, h]) // 16
                        csz = -(-ncols // gsplit)
                        for g0 in range(0, ncols, csz):
                            g1 = min(g0 + csz, ncols)
                            r = min(rows, g1 * P) - g0 * P
                            nc.gpsimd.dma_gather(
                                msgs[:, g0:g1, :],
                                (feat0 if h == 0 else feat1)[:, :],
                                idx_sb[
                                    :,
                                    s_off + g0 * 8 : s_off + g0 * 8 + r // 16,
                                ],
                                r,
                                r,
                                cfg.d,
                                single_packet=single_packet,
                                queue_num=gcount % nq,
                            )
                            gcount += 1
                        slabs[h] = msgs
                    if mode == "nocompute":
                        continue
                    psum_c = psa.tile([P, CHUNK], f32, tag="acc")
                    nc.tensor.matmul(
                        psum_c[:],
                        lhsT=zrow[:1, :],
                        rhs=z512[:1, :],
                        start=True,
                        stop=False,
                        skip_group_check=True,
                    )
                    for h in sorted(slabs):
                        t_lo, t_hi = plan.slab_tiles[(c, h)]
                        nt = t_hi - t_lo
                        if nt == 0:
                            continue
                        o_lo = t_lo
                        if mode == "constoh":
                            oh = oh0
                        else:
                            oh = ohpool.tile([P, nt, CELLW], f16, tag="oh")
                            nc.vector.tensor_tensor(
                                oh[:],
                                dst_sb[:, t_lo:t_hi]
                                .unsqueeze(2)
                                .broadcast_to([P, nt, CELLW]),
                                iota_sb[:]
                                .unsqueeze(1)
                                .broadcast_to([P, nt, CELLW]),
                                is_equal,
                            )
                        msgs = slabs[h]
                        for (cc, hh, col, base, t) in plan.tiles[t_lo:t_hi]:
                            nc.tensor.matmul(
                                psum_c[:, base : base + CELLW],
                                lhsT=msgs[:, col, :],
                                rhs=oh[:, t - o_lo, :],
                                start=False,
                                stop=(t == t_hi - 1 and h == max(slabs)),
                                skip_group_check=True,
                            )
                    if mode == "noepi":
                        continue
                    # Act copy can start as soon as the chunk's accumulation
                    # stops; the epilogue matmuls are deferred one chunk so
                    # PE never stalls waiting for the copy round-trip.
                    hc = eppool.tile([P, CHUNK], f16, tag="hc")
                    nc.scalar.copy(hc[:], psum_c[:])
                    pending.append((c, hc))
                    if len(pending) > 1:
                        emit_epilogue(*pending.pop(0))
                for ce in pending:
                    emit_epilogue(*ce)
                pending.clear()

            def emit_epilogue(c, hc):
                for j in range(CHUNK // P):
                    ps2 = psb.tile([P, P], f32, tag="p2")
                    nc.tensor.matmul(
                        ps2[:],
                        lhsT=hc[:, j * P : (j + 1) * P],
                        rhs=w_sb[:],
                        start=True,
                        stop=False,
                    )
                    nc.tensor.matmul(
                        ps2[:],
                        lhsT=ones_sb[:1, :],
                        rhs=b_sb[:1, :],
                        start=False,
                        stop=True,
                    )
                    ow = eppool.tile([P, P], f16, tag="ow")
                    nc.scalar.activation(
                        ow[:], ps2[:], mybir.ActivationFunctionType.Relu
                    )
                    r0 = c * CHUNK + j * P
                    nc.sync.dma_start(out[r0 : r0 + P, :], ow[:])

            pending = []

            if loop:
                with tc.For_i(0, trip_val):
                    emit_rep()
            else:
                for _rep in range(repeat):
                    emit_rep()
    nc.compile()
    return nc


def make_in_maps(cfg, feature, edge_src, edge_dst, W, b):
    feature = np.asarray(feature, np.float32)
    edge_src = np.asarray(edge_src, np.int32)
    edge_dst = np.asarray(edge_dst, np.int32)
    W = np.asarray(W, np.float32)
    b = np.asarray(b, np.float32)
    plan, idx16, dstw = _prepare(cfg, edge_src, edge_dst)
    f16 = np.ascontiguousarray(feature.astype(np.float16))
    feat0, feat1 = f16[: cfg.half], f16[cfg.half :]
    iota64 = np.ascontiguousarray(
        np.broadcast_to(np.arange(CELLW, dtype=np.float32), (P, CELLW))
    ).astype(np.float16)
    in_maps = [
        dict(
            feat0=feat0,
            feat1=feat1,
            idx16=np.ascontiguousarray(idx16[i]),
            dstw=np.ascontiguousarray(dstw[i]),
            iota64=iota64,
            wmat=W.astype(np.float16),
            bvec=b.astype(np.float16)[None, :],
        )
        for i in range(cfg.n_cores)
    ]
    return plan, in_maps


_BUILD_CACHE = {}


def run(feature, edge_src, edge_dst, W, b, cfg=CFG, trace=False, nq=4, **spmd_kwargs):
    plan, in_maps = make_in_maps(cfg, feature, edge_src, edge_dst, W, b)
    key = (plan.key(), nq)
    nc = _BUILD_CACHE.get(key)
    if nc is None:
        nc = _build(cfg, plan, nq=nq)
        _BUILD_CACHE[key] = nc
    res = run_bass_kernel_spmd(
        nc, in_maps, core_ids=list(range(cfg.n_cores)), trace=trace, **spmd_kwargs
    )
    out_full = np.empty((cfg.n_nodes, cfg.d), np.float32)
    for i in range(cfg.n_cores):
        o = np.asarray(res.results[i]["out"]).astype(np.float32)
        out_full[i * cfg.npc : (i + 1) * cfg.npc] = o[plan.slots[i]]
    return out_full, res


def kernel(**inputs):
    out, _ = run(
        inputs["feature"],
        inputs["edge_src"],
        inputs["edge_dst"],
        inputs["W"],
        inputs["b"],
    )
    return out
